# revision 7
# baseline (speedup 1.0000x reference)
"""Pairwise-distance retrieval kernel (nn_Cov) for 8 Trainium2 NeuronCores.

Computes, for seq [N, D] with 0/1 masks qvs_idx/sum_idx:
    A = seq * qvs, B = seq * sum
    dist = sqrt(max(a2_i + b2_j - 2 A@B^T, eps))    [N, N]
    norm = dist.mean();  mn_i = min over valid j of dist_ij
    out = (1 - min(mn, norm)/norm) @ weight + bias  [N, 1]

Sharding: rows of A (queries) split across 8 cores; B replicated.
Device computes per-row partial sums (for the global mean) and per-row
mins over the valid columns; the tiny coupling through the global scalar
`norm` is resolved on the host.

Device-side per (128-row, 512-col) tile:
  PSUM = b2_j - 2*G_ij    via a K=1 ones x b2 matmul (start=True) then
                          4 K=128 float32r matmuls (A pre-scaled by -2)
  DVE  : d2f = max(PSUM + a2_i, 0)         (tensor_scalar add+max)
  ACT  : sqrt(d2f) with accum_out -> per-row partial sums
  DVE  : reduce_min(d2f[:, :valid]) -> per-row partial mins
Valid (sum_idx=1) columns are permuted to the front on the host so the
min reduction covers a contiguous prefix — no masking op needed.
"""

import os
import sys

import numpy as np

for _p in ("/opt/trn_rl_repo",):
    if os.path.isdir(_p) and _p not in sys.path:
        sys.path.insert(0, _p)

import concourse.bacc as bacc
import concourse.bass as bass
import concourse.mybir as mybir
import concourse.tile as tile
from concourse.bass_utils import run_bass_kernel_spmd

N, D = 8192, 512
NCORES = 8
RPC = N // NCORES          # rows per core (1024)
MB = RPC // 128            # 128-row blocks per core (8)
CW = 512                   # column chunk width (one PSUM bank of fp32)
NCH = N // CW              # column chunks (16)
KCH = D // 128             # contraction chunks (4)
EPS = 1e-12

_BUILD_CACHE: dict = {}
LAST_RESULTS = None        # BassKernelResults of the most recent run


def _build(nvc_full: int, v_rem: int):
    """Build + compile the SPMD Bass program.

    nvc_full: number of full 512-wide column chunks that are entirely valid
    v_rem:    width of the partial boundary chunk (0 if none)
    """
    nc = bacc.Bacc("TRN2", target_bir_lowering=False)
    f32 = mybir.dt.float32
    f32r = mybir.dt.float32r
    AX = mybir.AxisListType.X
    OP = mybir.AluOpType

    at_d = nc.dram_tensor("at0", [KCH, 128, RPC], f32r, kind="ExternalInput")
    bt_d = nc.dram_tensor("bt0", [KCH, 128, N], f32r, kind="ExternalInput")
    b2_d = nc.dram_tensor("b20", [1, N], f32r, kind="ExternalInput")
    a2_d = nc.dram_tensor("a20", [128, MB], f32, kind="ExternalInput")
    rmin_d = nc.dram_tensor("rmin0", [128, MB], f32, kind="ExternalOutput")
    rsum_d = nc.dram_tensor("rsum0", [128, MB], f32, kind="ExternalOutput")

    nvc = nvc_full + (1 if v_rem else 0)
    assert nvc >= 1

    with tile.TileContext(nc) as tc:
        with (
            tc.tile_pool(name="big", bufs=1) as big,
            tc.tile_pool(name="work", bufs=4) as work,
            tc.tile_pool(name="acc", bufs=2) as accp,
            tc.tile_pool(name="psum", bufs=4, space="PSUM") as pp,
        ):
            bt_sb = []
            for k in range(KCH):
                t = big.tile([128, N], f32r, name=f"bt_sb{k}", tag=f"bt{k}")
                nc.sync.dma_start(t, bt_d[k])
                bt_sb.append(t)
            at_sb = []
            for k in range(KCH):
                t = big.tile([128, RPC], f32r, name=f"at_sb{k}", tag=f"at{k}")
                nc.sync.dma_start(t, at_d[k])
                at_sb.append(t)
            b2_sb = big.tile([1, N], f32r, name="b2_sb", tag="b2")
            nc.sync.dma_start(b2_sb, b2_d[:, :])
            a2_sb = big.tile([128, MB], f32, name="a2_sb", tag="a2")
            nc.sync.dma_start(a2_sb, a2_d[:, :])
            ones_f32 = big.tile([1, 128], f32, name="ones_f32", tag="ones")
            nc.vector.memset(ones_f32, 1.0)
            ones_sb = ones_f32.bitcast(f32r)
            rmin_sb = big.tile([128, MB], f32, name="rmin_sb", tag="rmin")
            rsum_sb = big.tile([128, MB], f32, name="rsum_sb", tag="rsum")

            for m in range(MB):
                sumbuf = accp.tile([128, NCH], f32, name="sumbuf", tag="sumbuf")
                minbuf = accp.tile([128, nvc], f32, name="minbuf", tag="minbuf")
                for n in range(NCH):
                    ps = pp.tile([128, CW], f32, name="ps", tag="ps")
                    nc.tensor.matmul(
                        ps, ones_sb, b2_sb[:, n * CW:(n + 1) * CW],
                        start=True, stop=False,
                    )
                    for k in range(KCH):
                        nc.tensor.matmul(
                            ps,
                            at_sb[k][:, m * 128:(m + 1) * 128],
                            bt_sb[k][:, n * CW:(n + 1) * CW],
                            start=False, stop=(k == KCH - 1),
                        )
                    d2f = work.tile([128, CW], f32, name="d2f", tag="d2f")
                    nc.vector.tensor_scalar(
                        d2f, ps, a2_sb[:, m:m + 1], 0.0, OP.add, OP.max
                    )
                    scr = work.tile([128, CW], f32, name="scr", tag="scr")
                    nc.scalar.activation(
                        scr, d2f, mybir.ActivationFunctionType.Sqrt,
                        accum_out=sumbuf[:, n:n + 1],
                    )
                    if n < nvc:
                        w = CW if n < nvc_full else v_rem
                        nc.vector.tensor_reduce(
                            minbuf[:, n:n + 1], d2f[:, :w], axis=AX, op=OP.min
                        )
                nc.vector.tensor_reduce(rsum_sb[:, m:m + 1], sumbuf, axis=AX, op=OP.add)
                nc.vector.tensor_reduce(rmin_sb[:, m:m + 1], minbuf, axis=AX, op=OP.min)
            nc.sync.dma_start(rmin_d[:, :], rmin_sb)
            nc.sync.dma_start(rsum_d[:, :], rsum_sb)

    nc.compile()
    return nc


def kernel(seq, weight, bias, qvs_idx, sum_idx):
    global LAST_RESULTS
    seq = np.asarray(seq, dtype=np.float32)
    weight = np.asarray(weight, dtype=np.float32)
    bias = np.asarray(bias, dtype=np.float32)
    qvs_idx = np.asarray(qvs_idx, dtype=np.int32)
    sum_idx = np.asarray(sum_idx, dtype=np.int32)

    mq = (qvs_idx[:, 0] != 0)
    ms = (sum_idx[:, 0] != 0)
    A = seq * mq[:, None].astype(np.float32)
    B = seq * ms[:, None].astype(np.float32)
    a2 = np.einsum("nd,nd->n", A, A, dtype=np.float32).astype(np.float32)
    s2 = np.einsum("nd,nd->n", seq, seq, dtype=np.float32).astype(np.float32)

    # Stable permutation: valid (sum_idx=1) columns first.
    perm = np.argsort(~ms, kind="stable")
    NV = int(ms.sum())
    Bp = B[perm]
    b2p = np.einsum("nd,nd->n", Bp, Bp, dtype=np.float32).astype(np.float32)

    nvc_full, v_rem = divmod(NV, CW)
    if nvc_full == 0 and v_rem == 0:
        # No valid columns: mn = inf -> clamps to norm -> simcov = 0.
        # Still run the device for the sum path via a 1-wide dummy min.
        nvc_full, v_rem = 0, 1

    key = (nvc_full, v_rem)
    if key not in _BUILD_CACHE:
        _BUILD_CACHE[key] = _build(nvc_full, v_rem)
    nc = _BUILD_CACHE[key]

    atT = np.ascontiguousarray((-2.0 * A).T)            # [D, N]
    btT = np.ascontiguousarray(Bp.T)                    # [D, N]
    bt_chunks = btT.reshape(KCH, 128, N)
    in_maps = []
    for c in range(NCORES):
        at_c = np.ascontiguousarray(
            atT[:, c * RPC:(c + 1) * RPC].reshape(KCH, 128, RPC)
        )
        a2_c = np.ascontiguousarray(
            a2[c * RPC:(c + 1) * RPC].reshape(MB, 128).T
        )
        in_maps.append({
            "at0": at_c,
            "bt0": bt_chunks,
            "b20": b2p.reshape(1, N),
            "a20": a2_c,
        })

    trace = bool(int(os.environ.get("NN_COV_TRACE", "0")))
    LAST_RESULTS = run_bass_kernel_spmd(
        nc, in_maps, core_ids=list(range(NCORES)), trace=trace
    )
    results = LAST_RESULTS.results

    row_min = np.empty(N, dtype=np.float32)
    row_sum = np.empty(N, dtype=np.float32)
    for c in range(NCORES):
        row_min[c * RPC:(c + 1) * RPC] = results[c]["rmin0"].T.reshape(RPC)
        row_sum[c * RPC:(c + 1) * RPC] = results[c]["rsum0"].T.reshape(RPC)

    norm = np.float32(row_sum.sum(dtype=np.float64) / (float(N) * float(N)))

    # Patch the diagonal with its exact value: d2_ii = (mq XOR ms) * s2_i.
    # (The device's diag entry carries matmul cancellation noise; the true
    # value is exact in closed form since A_i and B_i share seq_i.)
    d2_diag = np.where(mq ^ ms, s2, np.float32(0.0)).astype(np.float32)
    min_d2 = np.where(ms, np.minimum(row_min, d2_diag), row_min)
    if NV == 0:
        mn = np.full(N, np.inf, dtype=np.float32)
    else:
        mn = np.sqrt(np.maximum(min_d2, np.float32(EPS)))
    mn = np.minimum(mn, norm)
    simcov = (np.float32(1.0) - mn / norm).astype(np.float32)[:, None]
    out = simcov @ weight + bias[None, :]
    return out.astype(np.float32)


# revision 12
# speedup vs baseline: 2.2013x; 2.2013x over previous
"""Pairwise-distance retrieval kernel (nn_Cov) for 8 Trainium2 NeuronCores.

Computes, for seq [N, D] with 0/1 masks qvs_idx/sum_idx:
    A = seq * qvs, B = seq * sum
    dist = sqrt(max(a2_i + b2_j - 2 A@B^T, eps))    [N, N]
    norm = dist.mean();  mn_i = min over valid j of dist_ij
    out = (1 - min(mn, norm)/norm) @ weight + bias  [N, 1]

Sharding: rows of A (queries) split across 8 cores; B replicated.
Device computes per-row partial sums (for the global mean) and per-row
mins over the valid columns; the tiny coupling through the global scalar
`norm` is resolved on the host.

Device-side per (128-row, 512-col) tile:
  PSUM = b2_j - 2*G_ij    via a K=1 ones x b2 matmul (start=True) then
                          4 K=128 float32r matmuls (A pre-scaled by -2)
  DVE  : d2f = max(PSUM + a2_i, 0)         (tensor_scalar add+max)
  ACT  : sqrt(d2f) with accum_out -> per-row partial sums
  DVE  : reduce_min(d2f[:, :valid]) -> per-row partial mins
Valid (sum_idx=1) columns are permuted to the front on the host so the
min reduction covers a contiguous prefix — no masking op needed.
"""

import os
import sys

import numpy as np

for _p in ("/opt/trn_rl_repo",):
    if os.path.isdir(_p) and _p not in sys.path:
        sys.path.insert(0, _p)

import concourse.bacc as bacc
import concourse.bass as bass
import concourse.mybir as mybir
import concourse.tile as tile
from concourse.bass_utils import run_bass_kernel_spmd

N, D = 8192, 512
NCORES = 8
RPC = N // NCORES          # rows per core (1024)
MB = RPC // 128            # 128-row blocks per core (8)
CW = 512                   # column chunk width (one PSUM bank of fp32)
NCH = N // CW              # column chunks (16)
KCH = D // 128             # contraction chunks (4)
EPS = 1e-12

_BUILD_CACHE: dict = {}
LAST_RESULTS = None        # BassKernelResults of the most recent run


PAIRW = 2 * CW             # DVE/ACT operate on two banks at once (1024)
NPAIR = NCH // 2           # column pairs (8)
GRP = 4                    # column chunks per weight-reuse group


def _build(nvc_full: int, v_rem: int):
    """Build + compile the SPMD Bass program.

    nvc_full: number of full 512-wide column chunks that are entirely valid
    v_rem:    width of the partial boundary chunk (0 if none)
    """
    nc = bacc.Bacc("TRN2", target_bir_lowering=False)
    f32 = mybir.dt.float32
    bf16 = mybir.dt.bfloat16
    AX = mybir.AxisListType.X
    OP = mybir.AluOpType

    at_d = nc.dram_tensor("at0", [KCH, 128, RPC], bf16, kind="ExternalInput")
    bt_d = nc.dram_tensor("bt0", [KCH, 128, N], bf16, kind="ExternalInput")
    b2_d = nc.dram_tensor("b20", [1, N], bf16, kind="ExternalInput")
    a2_d = nc.dram_tensor("a20", [128, MB], f32, kind="ExternalInput")
    rmin_d = nc.dram_tensor("rmin0", [128, MB], f32, kind="ExternalOutput")
    rsum_d = nc.dram_tensor("rsum0", [128, MB], f32, kind="ExternalOutput")

    NV = nvc_full * CW + v_rem       # number of valid (leading) columns
    npv = (NV + PAIRW - 1) // PAIRW  # pairs that intersect the valid prefix
    assert npv >= 1

    with tile.TileContext(nc) as tc:
        with (
            tc.tile_pool(name="big", bufs=1) as big,
            tc.tile_pool(name="work", bufs=4) as work,
            tc.tile_pool(name="acc", bufs=2) as accp,
            tc.tile_pool(name="psum", bufs=4, space="PSUM") as pp,
        ):
            bt_sb = []
            for k in range(KCH):
                t = big.tile([128, N], bf16, name=f"bt_sb{k}", tag=f"bt{k}")
                nc.sync.dma_start(t, bt_d[k])
                bt_sb.append(t)
            at_sb = []
            for k in range(KCH):
                t = big.tile([128, RPC], bf16, name=f"at_sb{k}", tag=f"at{k}")
                nc.sync.dma_start(t, at_d[k])
                at_sb.append(t)
            b2_sb = big.tile([1, N], bf16, name="b2_sb", tag="b2")
            nc.sync.dma_start(b2_sb, b2_d[:, :])
            a2_sb = big.tile([128, MB], f32, name="a2_sb", tag="a2")
            nc.sync.dma_start(a2_sb, a2_d[:, :])
            ones_sb = big.tile([1, 128], bf16, name="ones_sb", tag="ones")
            nc.vector.memset(ones_sb, 1.0)
            rmin_sb = big.tile([128, MB], f32, name="rmin_sb", tag="rmin")
            rsum_sb = big.tile([128, MB], f32, name="rsum_sb", tag="rsum")

            for m in range(MB):
                sumbuf = accp.tile([128, NPAIR], f32, name="sumbuf", tag="sumbuf")
                minbuf = accp.tile([128, npv], f32, name="minbuf", tag="minbuf")
                for g in range(NCH // GRP):
                    # two 1024-wide psum tiles per group of 4 column chunks
                    pairs = []
                    for h in range(GRP // 2):
                        ps = pp.tile([128, PAIRW], f32, name="ps", tag="ps")
                        pairs.append(ps)
                    halves = [
                        (pairs[h // 2], (h % 2) * CW, g * GRP + h)
                        for h in range(GRP)
                    ]
                    # prefill each 512-half with b2 via a K=1 ones matmul
                    for ps, off, n in halves:
                        nc.tensor.matmul(
                            ps[:, off:off + CW], ones_sb,
                            b2_sb[:, n * CW:(n + 1) * CW],
                            start=True, stop=False,
                        )
                    # k-outer so 4 consecutive matmuls share the same lhsT
                    for k in range(KCH):
                        for ps, off, n in halves:
                            nc.tensor.matmul(
                                ps[:, off:off + CW],
                                at_sb[k][:, m * 128:(m + 1) * 128],
                                bt_sb[k][:, n * CW:(n + 1) * CW],
                                start=False, stop=(k == KCH - 1),
                            )
                    for h2 in range(GRP // 2):
                        ps = pairs[h2]
                        p = g * (GRP // 2) + h2     # pair index 0..NPAIR-1
                        n0 = 2 * p                  # first chunk of the pair
                        d2f = work.tile([128, PAIRW], f32, name="d2f", tag="d2f")
                        nc.vector.tensor_scalar(
                            d2f, ps, a2_sb[:, m:m + 1], 0.0, OP.add, OP.max
                        )
                        scr = work.tile([128, PAIRW], f32, name="scr", tag="scr")
                        nc.scalar.activation(
                            scr, d2f, mybir.ActivationFunctionType.Sqrt,
                            accum_out=sumbuf[:, p:p + 1],
                        )
                        # min over the valid prefix covered by this pair
                        lo = n0 * CW
                        hi = min(NV, lo + PAIRW)
                        if hi > lo:
                            nc.vector.tensor_reduce(
                                minbuf[:, p:p + 1], d2f[:, :hi - lo],
                                axis=AX, op=OP.min,
                            )
                nc.vector.tensor_reduce(rsum_sb[:, m:m + 1], sumbuf, axis=AX, op=OP.add)
                nc.vector.tensor_reduce(rmin_sb[:, m:m + 1], minbuf, axis=AX, op=OP.min)
            nc.sync.dma_start(rmin_d[:, :], rmin_sb)
            nc.sync.dma_start(rsum_d[:, :], rsum_sb)

    nc.compile()
    return nc


def kernel(seq, weight, bias, qvs_idx, sum_idx):
    global LAST_RESULTS
    seq = np.asarray(seq, dtype=np.float32)
    weight = np.asarray(weight, dtype=np.float32)
    bias = np.asarray(bias, dtype=np.float32)
    qvs_idx = np.asarray(qvs_idx, dtype=np.int32)
    sum_idx = np.asarray(sum_idx, dtype=np.int32)

    mq = (qvs_idx[:, 0] != 0)
    ms = (sum_idx[:, 0] != 0)
    A = seq * mq[:, None].astype(np.float32)
    B = seq * ms[:, None].astype(np.float32)
    a2 = np.einsum("nd,nd->n", A, A, dtype=np.float32).astype(np.float32)
    s2 = np.einsum("nd,nd->n", seq, seq, dtype=np.float32).astype(np.float32)

    # Stable permutation: valid (sum_idx=1) columns first.
    perm = np.argsort(~ms, kind="stable")
    NV = int(ms.sum())
    Bp = B[perm]
    b2p = np.einsum("nd,nd->n", Bp, Bp, dtype=np.float32).astype(np.float32)

    nvc_full, v_rem = divmod(NV, CW)
    if nvc_full == 0 and v_rem == 0:
        # No valid columns: mn = inf -> clamps to norm -> simcov = 0.
        # Still run the device for the sum path via a 1-wide dummy min.
        nvc_full, v_rem = 0, 1

    key = (nvc_full, v_rem)
    if key not in _BUILD_CACHE:
        _BUILD_CACHE[key] = _build(nvc_full, v_rem)
    nc = _BUILD_CACHE[key]

    import ml_dtypes

    bf16 = ml_dtypes.bfloat16
    atT = np.ascontiguousarray((-2.0 * A).T.astype(bf16))   # [D, N]
    btT = np.ascontiguousarray(Bp.T.astype(bf16))           # [D, N]
    b2bf = b2p.astype(bf16)
    bt_chunks = btT.reshape(KCH, 128, N)
    in_maps = []
    for c in range(NCORES):
        at_c = np.ascontiguousarray(
            atT[:, c * RPC:(c + 1) * RPC].reshape(KCH, 128, RPC)
        )
        a2_c = np.ascontiguousarray(
            a2[c * RPC:(c + 1) * RPC].reshape(MB, 128).T
        )
        in_maps.append({
            "at0": at_c,
            "bt0": bt_chunks,
            "b20": b2bf.reshape(1, N),
            "a20": a2_c,
        })

    trace = bool(int(os.environ.get("NN_COV_TRACE", "0")))
    LAST_RESULTS = run_bass_kernel_spmd(
        nc, in_maps, core_ids=list(range(NCORES)), trace=trace
    )
    results = LAST_RESULTS.results

    row_min = np.empty(N, dtype=np.float32)
    row_sum = np.empty(N, dtype=np.float32)
    for c in range(NCORES):
        row_min[c * RPC:(c + 1) * RPC] = results[c]["rmin0"].T.reshape(RPC)
        row_sum[c * RPC:(c + 1) * RPC] = results[c]["rsum0"].T.reshape(RPC)

    norm = np.float32(row_sum.sum(dtype=np.float64) / (float(N) * float(N)))

    # Patch the diagonal with its exact value: d2_ii = (mq XOR ms) * s2_i.
    # (The device's diag entry carries matmul cancellation noise; the true
    # value is exact in closed form since A_i and B_i share seq_i.)
    d2_diag = np.where(mq ^ ms, s2, np.float32(0.0)).astype(np.float32)
    min_d2 = np.where(ms, np.minimum(row_min, d2_diag), row_min)
    if NV == 0:
        mn = np.full(N, np.inf, dtype=np.float32)
    else:
        mn = np.sqrt(np.maximum(min_d2, np.float32(EPS)))
    mn = np.minimum(mn, norm)
    simcov = (np.float32(1.0) - mn / norm).astype(np.float32)[:, None]
    out = simcov @ weight + bias[None, :]
    return out.astype(np.float32)


# revision 15
# speedup vs baseline: 3.9340x; 1.7871x over previous
"""Pairwise-distance retrieval kernel (nn_Cov) for 8 Trainium2 NeuronCores.

Computes, for seq [N, D] with 0/1 masks qvs_idx/sum_idx:
    A = seq * qvs, B = seq * sum
    dist = sqrt(max(a2_i + b2_j - 2 A@B^T, eps))    [N, N]
    norm = dist.mean();  mn_i = min over valid j of dist_ij
    out = (1 - min(mn, norm)/norm) @ weight + bias  [N, 1]

Sharding: rows of A (queries) split across 8 cores; B replicated.
Device computes per-row partial sums (for the global mean) and per-row
mins over the valid columns; the tiny coupling through the global scalar
`norm` is resolved on the host.

Device-side per (128-row, 512-col) tile:
  PSUM = b2_j - 2*G_ij    via a K=1 ones x b2 matmul (start=True) then
                          4 K=128 float32r matmuls (A pre-scaled by -2)
  DVE  : d2f = max(PSUM + a2_i, 0)         (tensor_scalar add+max)
  ACT  : sqrt(d2f) with accum_out -> per-row partial sums
  DVE  : reduce_min(d2f[:, :valid]) -> per-row partial mins
Valid (sum_idx=1) columns are permuted to the front on the host so the
min reduction covers a contiguous prefix — no masking op needed.
"""

import os
import sys

import numpy as np

for _p in ("/opt/trn_rl_repo",):
    if os.path.isdir(_p) and _p not in sys.path:
        sys.path.insert(0, _p)

import concourse.bacc as bacc
import concourse.bass as bass
import concourse.mybir as mybir
import concourse.tile as tile
from concourse.bass_utils import run_bass_kernel_spmd

N, D = 8192, 512
NCORES = 8
RPC = N // NCORES          # rows per core (1024)
MB = RPC // 128            # 128-row blocks per core (8)
CW = 512                   # column chunk width (one PSUM bank of fp32)
NCH = N // CW              # column chunks (16)
KCH = D // 128             # contraction chunks (4)
EPS = 1e-12

_BUILD_CACHE: dict = {}
LAST_RESULTS = None        # BassKernelResults of the most recent run


PAIRW = 2 * CW             # DVE/ACT operate on two banks at once (1024)
NPAIR = NCH // 2           # column pairs (8)
GRP = 4                    # column chunks per weight-reuse group


def _build(nvc_full: int, v_rem: int):
    """Build + compile the SPMD Bass program.

    The device only processes the leading NVC = ceil(NV/512) column chunks
    (valid columns are permuted to the front on the host). Columns beyond
    that have B == 0 exactly, so dist_ij = sqrt(a2_i) — the host adds their
    contribution to the row sums in closed form.

    nvc_full: number of full 512-wide column chunks that are entirely valid
    v_rem:    width of the partial boundary chunk (0 if none)
    """
    nc = bacc.Bacc("TRN2", target_bir_lowering=False)
    f32 = mybir.dt.float32
    bf16 = mybir.dt.bfloat16
    AX = mybir.AxisListType.X
    OP = mybir.AluOpType

    NV = nvc_full * CW + v_rem       # number of valid (leading) columns
    NVC = nvc_full + (1 if v_rem else 0)  # processed column chunks
    NPW = NVC * CW                   # processed columns (device-covered)
    npairs = (NVC + 1) // 2          # 1024-wide pairs (last may be 512 wide)
    npv = (NV + PAIRW - 1) // PAIRW  # pairs intersecting the valid prefix
    assert npv >= 1

    at_d = nc.dram_tensor("at0", [KCH, 128, RPC], bf16, kind="ExternalInput")
    bt_d = nc.dram_tensor("bt0", [KCH, 128, NPW], bf16, kind="ExternalInput")
    b2_d = nc.dram_tensor("b20", [1, NPW], bf16, kind="ExternalInput")
    a2_d = nc.dram_tensor("a20", [128, MB], f32, kind="ExternalInput")
    rmin_d = nc.dram_tensor("rmin0", [128, MB], f32, kind="ExternalOutput")
    rsum_d = nc.dram_tensor("rsum0", [128, MB], f32, kind="ExternalOutput")

    groups = [list(range(g, min(g + GRP, NVC))) for g in range(0, NVC, GRP)]

    with tile.TileContext(nc) as tc:
        with (
            tc.tile_pool(name="big", bufs=1) as big,
            tc.tile_pool(name="work", bufs=4) as work,
            tc.tile_pool(name="acc", bufs=2) as accp,
            tc.tile_pool(name="psum", bufs=4, space="PSUM") as pp,
        ):
            b2_sb = big.tile([1, NPW], bf16, name="b2_sb", tag="b2")
            nc.sync.dma_start(b2_sb, b2_d[:, :])
            a2_sb = big.tile([128, MB], f32, name="a2_sb", tag="a2")
            nc.sync.dma_start(a2_sb, a2_d[:, :])
            ones_sb = big.tile([1, 128], bf16, name="ones_sb", tag="ones")
            nc.vector.memset(ones_sb, 1.0)
            at_sb = []
            for k in range(KCH):
                t = big.tile([128, RPC], bf16, name=f"at_sb{k}", tag=f"at{k}")
                nc.sync.dma_start(t, at_d[k])
                at_sb.append(t)
            # bt split per column group so compute starts after piece 0
            bt_sb = [
                big.tile([128, NPW], bf16, name=f"bt_sb{k}", tag=f"bt{k}")
                for k in range(KCH)
            ]
            for grp in groups:
                lo, hi = grp[0] * CW, (grp[-1] + 1) * CW
                for k in range(KCH):
                    nc.sync.dma_start(bt_sb[k][:, lo:hi], bt_d[k][:, lo:hi])
            rmin_sb = big.tile([128, MB], f32, name="rmin_sb", tag="rmin")
            rsum_sb = big.tile([128, MB], f32, name="rsum_sb", tag="rsum")

            for m in range(MB):
                sumbuf = accp.tile([128, npairs], f32, name="sumbuf", tag="sumbuf")
                minbuf = accp.tile([128, npv], f32, name="minbuf", tag="minbuf")
                for grp in groups:
                    # pair consecutive chunks into up-to-1024-wide psum tiles
                    pair_chunks = [grp[i:i + 2] for i in range(0, len(grp), 2)]
                    pairs = []
                    for pc in pair_chunks:
                        ps = pp.tile([128, PAIRW], f32, name="ps", tag="ps")
                        pairs.append(ps)
                    halves = []
                    for ps, pc in zip(pairs, pair_chunks):
                        for i, n in enumerate(pc):
                            halves.append((ps, i * CW, n))
                    # prefill each 512-half with b2 via a K=1 ones matmul
                    for ps, off, n in halves:
                        nc.tensor.matmul(
                            ps[:, off:off + CW], ones_sb,
                            b2_sb[:, n * CW:(n + 1) * CW],
                            start=True, stop=False,
                        )
                    # k-outer so consecutive matmuls share the same lhsT
                    for k in range(KCH):
                        for ps, off, n in halves:
                            nc.tensor.matmul(
                                ps[:, off:off + CW],
                                at_sb[k][:, m * 128:(m + 1) * 128],
                                bt_sb[k][:, n * CW:(n + 1) * CW],
                                start=False, stop=(k == KCH - 1),
                            )
                    for ps, pc in zip(pairs, pair_chunks):
                        p = pc[0] // 2              # pair index
                        w = len(pc) * CW            # 1024, or 512 on the tail
                        d2f = work.tile([128, PAIRW], f32, name="d2f", tag="d2f")
                        nc.vector.tensor_scalar(
                            d2f[:, :w], ps[:, :w], a2_sb[:, m:m + 1], 0.0,
                            OP.add, OP.max,
                        )
                        scr = work.tile([128, PAIRW], f32, name="scr", tag="scr")
                        nc.scalar.activation(
                            scr[:, :w], d2f[:, :w],
                            mybir.ActivationFunctionType.Sqrt,
                            accum_out=sumbuf[:, p:p + 1],
                        )
                        # min over the valid prefix covered by this pair
                        lo = pc[0] * CW
                        hi = min(NV, lo + w)
                        if hi > lo:
                            nc.vector.tensor_reduce(
                                minbuf[:, p:p + 1], d2f[:, :hi - lo],
                                axis=AX, op=OP.min,
                            )
                nc.vector.tensor_reduce(rsum_sb[:, m:m + 1], sumbuf, axis=AX, op=OP.add)
                nc.vector.tensor_reduce(rmin_sb[:, m:m + 1], minbuf, axis=AX, op=OP.min)
            nc.sync.dma_start(rmin_d[:, :], rmin_sb)
            nc.sync.dma_start(rsum_d[:, :], rsum_sb)

    nc.compile()
    return nc


def kernel(seq, weight, bias, qvs_idx, sum_idx):
    global LAST_RESULTS
    seq = np.asarray(seq, dtype=np.float32)
    weight = np.asarray(weight, dtype=np.float32)
    bias = np.asarray(bias, dtype=np.float32)
    qvs_idx = np.asarray(qvs_idx, dtype=np.int32)
    sum_idx = np.asarray(sum_idx, dtype=np.int32)

    mq = (qvs_idx[:, 0] != 0)
    ms = (sum_idx[:, 0] != 0)
    A = seq * mq[:, None].astype(np.float32)
    B = seq * ms[:, None].astype(np.float32)
    a2 = np.einsum("nd,nd->n", A, A, dtype=np.float32).astype(np.float32)
    s2 = np.einsum("nd,nd->n", seq, seq, dtype=np.float32).astype(np.float32)

    # Stable permutation: valid (sum_idx=1) columns first.
    perm = np.argsort(~ms, kind="stable")
    NV = int(ms.sum())
    Bp = B[perm]
    b2p = np.einsum("nd,nd->n", Bp, Bp, dtype=np.float32).astype(np.float32)

    nvc_full, v_rem = divmod(NV, CW)
    if nvc_full == 0 and v_rem == 0:
        # No valid columns: mn = inf -> clamps to norm -> simcov = 0.
        # Still run the device for the sum path via a 1-wide dummy min.
        nvc_full, v_rem = 0, 1

    key = (nvc_full, v_rem)
    if key not in _BUILD_CACHE:
        _BUILD_CACHE[key] = _build(nvc_full, v_rem)
    nc = _BUILD_CACHE[key]

    import ml_dtypes

    bf16 = ml_dtypes.bfloat16
    NVC = nvc_full + (1 if v_rem else 0)
    NPW = NVC * CW
    atT = np.ascontiguousarray((-2.0 * A).T.astype(bf16))        # [D, N]
    btT = np.ascontiguousarray(Bp[:NPW].T.astype(bf16))          # [D, NPW]
    b2bf = b2p[:NPW].astype(bf16)
    bt_chunks = btT.reshape(KCH, 128, NPW)
    in_maps = []
    for c in range(NCORES):
        at_c = np.ascontiguousarray(
            atT[:, c * RPC:(c + 1) * RPC].reshape(KCH, 128, RPC)
        )
        a2_c = np.ascontiguousarray(
            a2[c * RPC:(c + 1) * RPC].reshape(MB, 128).T
        )
        in_maps.append({
            "at0": at_c,
            "bt0": bt_chunks,
            "b20": b2bf.reshape(1, NPW),
            "a20": a2_c,
        })

    trace = bool(int(os.environ.get("NN_COV_TRACE", "0")))
    LAST_RESULTS = run_bass_kernel_spmd(
        nc, in_maps, core_ids=list(range(NCORES)), trace=trace
    )
    results = LAST_RESULTS.results

    row_min = np.empty(N, dtype=np.float32)
    row_sum = np.empty(N, dtype=np.float32)
    for c in range(NCORES):
        row_min[c * RPC:(c + 1) * RPC] = results[c]["rmin0"].T.reshape(RPC)
        row_sum[c * RPC:(c + 1) * RPC] = results[c]["rsum0"].T.reshape(RPC)

    # Columns beyond the processed prefix have B == 0 exactly:
    # dist_ij = sqrt(max(a2_i, eps)). Add them in closed form.
    n_rest = N - NPW
    if n_rest > 0:
        row_sum = row_sum + np.float32(n_rest) * np.sqrt(
            np.maximum(a2, np.float32(EPS))
        ).astype(np.float32)

    norm = np.float32(row_sum.sum(dtype=np.float64) / (float(N) * float(N)))

    # Patch the diagonal with its exact value: d2_ii = (mq XOR ms) * s2_i.
    # (The device's diag entry carries matmul cancellation noise; the true
    # value is exact in closed form since A_i and B_i share seq_i.)
    d2_diag = np.where(mq ^ ms, s2, np.float32(0.0)).astype(np.float32)
    min_d2 = np.where(ms, np.minimum(row_min, d2_diag), row_min)
    if NV == 0:
        mn = np.full(N, np.inf, dtype=np.float32)
    else:
        mn = np.sqrt(np.maximum(min_d2, np.float32(EPS)))
    mn = np.minimum(mn, norm)
    simcov = (np.float32(1.0) - mn / norm).astype(np.float32)[:, None]
    out = simcov @ weight + bias[None, :]
    return out.astype(np.float32)


# revision 16
# speedup vs baseline: 4.0893x; 1.0395x over previous
"""Pairwise-distance retrieval kernel (nn_Cov) for 8 Trainium2 NeuronCores.

Computes, for seq [N, D] with 0/1 masks qvs_idx/sum_idx:
    A = seq * qvs, B = seq * sum
    dist = sqrt(max(a2_i + b2_j - 2 A@B^T, eps))    [N, N]
    norm = dist.mean();  mn_i = min over valid j of dist_ij
    out = (1 - min(mn, norm)/norm) @ weight + bias  [N, 1]

Sharding: rows of A (queries) split across 8 cores; B replicated.
Device computes per-row partial sums (for the global mean) and per-row
mins over the valid columns; the tiny coupling through the global scalar
`norm` is resolved on the host.

Device-side per (128-row, 512-col) tile:
  PSUM = b2_j - 2*G_ij    via a K=1 ones x b2 matmul (start=True) then
                          4 K=128 float32r matmuls (A pre-scaled by -2)
  DVE  : d2f = max(PSUM + a2_i, 0)         (tensor_scalar add+max)
  ACT  : sqrt(d2f) with accum_out -> per-row partial sums
  DVE  : reduce_min(d2f[:, :valid]) -> per-row partial mins
Valid (sum_idx=1) columns are permuted to the front on the host so the
min reduction covers a contiguous prefix — no masking op needed.
"""

import os
import sys

import numpy as np

for _p in ("/opt/trn_rl_repo",):
    if os.path.isdir(_p) and _p not in sys.path:
        sys.path.insert(0, _p)

import concourse.bacc as bacc
import concourse.bass as bass
import concourse.mybir as mybir
import concourse.tile as tile
from concourse.bass_utils import run_bass_kernel_spmd

N, D = 8192, 512
NCORES = 8
RPC = N // NCORES          # rows per core (1024)
MB = RPC // 128            # 128-row blocks per core (8)
CW = 512                   # column chunk width (one PSUM bank of fp32)
NCH = N // CW              # column chunks (16)
KCH = D // 128             # contraction chunks (4)
EPS = 1e-12

_BUILD_CACHE: dict = {}
LAST_RESULTS = None        # BassKernelResults of the most recent run


PAIRW = 2 * CW             # DVE/ACT operate on two banks at once (1024)
NPAIR = NCH // 2           # column pairs (8)
GRP = 4                    # column chunks per weight-reuse group


def _build(nvc_full: int, v_rem: int):
    """Build + compile the SPMD Bass program.

    The device only processes the leading NVC = ceil(NV/512) column chunks
    (valid columns are permuted to the front on the host). Columns beyond
    that have B == 0 exactly, so dist_ij = sqrt(a2_i) — the host adds their
    contribution to the row sums in closed form.

    nvc_full: number of full 512-wide column chunks that are entirely valid
    v_rem:    width of the partial boundary chunk (0 if none)
    """
    nc = bacc.Bacc("TRN2", target_bir_lowering=False)
    f32 = mybir.dt.float32
    bf16 = mybir.dt.bfloat16
    AX = mybir.AxisListType.X
    OP = mybir.AluOpType

    NV = nvc_full * CW + v_rem       # number of valid (leading) columns
    NVC = nvc_full + (1 if v_rem else 0)  # processed column chunks
    NPW = NVC * CW                   # processed columns (device-covered)
    npairs = (NVC + 1) // 2          # 1024-wide pairs (last may be 512 wide)
    npv = (NV + PAIRW - 1) // PAIRW  # pairs intersecting the valid prefix
    assert npv >= 1

    at_d = nc.dram_tensor("at0", [KCH, 128, RPC], bf16, kind="ExternalInput")
    bt_d = nc.dram_tensor("bt0", [KCH, 128, NPW], bf16, kind="ExternalInput")
    b2_d = nc.dram_tensor("b20", [1, NPW], bf16, kind="ExternalInput")
    a2_d = nc.dram_tensor("a20", [128, MB], f32, kind="ExternalInput")
    rmin_d = nc.dram_tensor("rmin0", [128, MB], f32, kind="ExternalOutput")
    rsum_d = nc.dram_tensor("rsum0", [128, MB], f32, kind="ExternalOutput")

    groups = [list(range(g, min(g + GRP, NVC))) for g in range(0, NVC, GRP)]

    with tile.TileContext(nc) as tc:
        with (
            tc.tile_pool(name="big", bufs=1) as big,
            tc.tile_pool(name="work", bufs=4) as work,
            tc.tile_pool(name="acc", bufs=2) as accp,
            tc.tile_pool(name="psum", bufs=4, space="PSUM") as pp,
        ):
            b2_sb = big.tile([1, NPW], bf16, name="b2_sb", tag="b2")
            nc.sync.dma_start(b2_sb, b2_d[:, :])
            a2_sb = big.tile([128, MB], f32, name="a2_sb", tag="a2")
            nc.sync.dma_start(a2_sb, a2_d[:, :])
            ones_sb = big.tile([1, 128], bf16, name="ones_sb", tag="ones")
            nc.vector.memset(ones_sb, 1.0)
            at_sb = []
            for k in range(KCH):
                t = big.tile([128, RPC], bf16, name=f"at_sb{k}", tag=f"at{k}")
                nc.sync.dma_start(t, at_d[k])
                at_sb.append(t)
            # bt split per column group so compute starts after piece 0
            bt_sb = [
                big.tile([128, NPW], bf16, name=f"bt_sb{k}", tag=f"bt{k}")
                for k in range(KCH)
            ]
            for grp in groups:
                lo, hi = grp[0] * CW, (grp[-1] + 1) * CW
                for k in range(KCH):
                    nc.sync.dma_start(bt_sb[k][:, lo:hi], bt_d[k][:, lo:hi])
            rmin_sb = big.tile([128, MB], f32, name="rmin_sb", tag="rmin")
            rsum_sb = big.tile([128, MB], f32, name="rsum_sb", tag="rsum")

            for m in range(MB):
                sumbuf = accp.tile([128, npairs], f32, name="sumbuf", tag="sumbuf")
                minbuf = accp.tile([128, npv], f32, name="minbuf", tag="minbuf")
                for grp in groups:
                    # pair consecutive chunks into up-to-1024-wide psum tiles
                    pair_chunks = [grp[i:i + 2] for i in range(0, len(grp), 2)]
                    pairs = []
                    for pc in pair_chunks:
                        ps = pp.tile([128, PAIRW], f32, name="ps", tag="ps")
                        pairs.append(ps)
                    halves = []
                    for ps, pc in zip(pairs, pair_chunks):
                        for i, n in enumerate(pc):
                            halves.append((ps, i * CW, n))
                    # prefill each 512-half with b2 via a K=1 ones matmul
                    for ps, off, n in halves:
                        nc.tensor.matmul(
                            ps[:, off:off + CW], ones_sb,
                            b2_sb[:, n * CW:(n + 1) * CW],
                            start=True, stop=False,
                        )
                    # k-outer so consecutive matmuls share the same lhsT
                    for k in range(KCH):
                        for ps, off, n in halves:
                            nc.tensor.matmul(
                                ps[:, off:off + CW],
                                at_sb[k][:, m * 128:(m + 1) * 128],
                                bt_sb[k][:, n * CW:(n + 1) * CW],
                                start=False, stop=(k == KCH - 1),
                            )
                    for ps, pc in zip(pairs, pair_chunks):
                        p = pc[0] // 2              # pair index
                        w = len(pc) * CW            # 1024, or 512 on the tail
                        d2f = work.tile([128, PAIRW], f32, name="d2f", tag="d2f")
                        # Floor+bias: max(psum + a2, 0). Split across engines
                        # to balance DVE vs ACT load (Relu(x + bias) on the
                        # scalar engine is the same function).
                        if w == PAIRW and p % 2 == 0:
                            nc.scalar.activation(
                                d2f[:, :w], ps[:, :w],
                                mybir.ActivationFunctionType.Relu,
                                bias=a2_sb[:, m:m + 1],
                            )
                        else:
                            nc.vector.tensor_scalar(
                                d2f[:, :w], ps[:, :w], a2_sb[:, m:m + 1], 0.0,
                                OP.add, OP.max,
                            )
                        scr = work.tile([128, PAIRW], f32, name="scr", tag="scr")
                        nc.scalar.activation(
                            scr[:, :w], d2f[:, :w],
                            mybir.ActivationFunctionType.Sqrt,
                            accum_out=sumbuf[:, p:p + 1],
                        )
                        # min over the valid prefix covered by this pair
                        lo = pc[0] * CW
                        hi = min(NV, lo + w)
                        if hi > lo:
                            nc.vector.tensor_reduce(
                                minbuf[:, p:p + 1], d2f[:, :hi - lo],
                                axis=AX, op=OP.min,
                            )
                nc.vector.tensor_reduce(rsum_sb[:, m:m + 1], sumbuf, axis=AX, op=OP.add)
                nc.vector.tensor_reduce(rmin_sb[:, m:m + 1], minbuf, axis=AX, op=OP.min)
            nc.sync.dma_start(rmin_d[:, :], rmin_sb)
            nc.sync.dma_start(rsum_d[:, :], rsum_sb)

    nc.compile()
    return nc


def kernel(seq, weight, bias, qvs_idx, sum_idx):
    global LAST_RESULTS
    seq = np.asarray(seq, dtype=np.float32)
    weight = np.asarray(weight, dtype=np.float32)
    bias = np.asarray(bias, dtype=np.float32)
    qvs_idx = np.asarray(qvs_idx, dtype=np.int32)
    sum_idx = np.asarray(sum_idx, dtype=np.int32)

    mq = (qvs_idx[:, 0] != 0)
    ms = (sum_idx[:, 0] != 0)
    A = seq * mq[:, None].astype(np.float32)
    B = seq * ms[:, None].astype(np.float32)
    a2 = np.einsum("nd,nd->n", A, A, dtype=np.float32).astype(np.float32)
    s2 = np.einsum("nd,nd->n", seq, seq, dtype=np.float32).astype(np.float32)

    # Stable permutation: valid (sum_idx=1) columns first.
    perm = np.argsort(~ms, kind="stable")
    NV = int(ms.sum())
    Bp = B[perm]
    b2p = np.einsum("nd,nd->n", Bp, Bp, dtype=np.float32).astype(np.float32)

    nvc_full, v_rem = divmod(NV, CW)
    if nvc_full == 0 and v_rem == 0:
        # No valid columns: mn = inf -> clamps to norm -> simcov = 0.
        # Still run the device for the sum path via a 1-wide dummy min.
        nvc_full, v_rem = 0, 1

    key = (nvc_full, v_rem)
    if key not in _BUILD_CACHE:
        _BUILD_CACHE[key] = _build(nvc_full, v_rem)
    nc = _BUILD_CACHE[key]

    import ml_dtypes

    bf16 = ml_dtypes.bfloat16
    NVC = nvc_full + (1 if v_rem else 0)
    NPW = NVC * CW
    atT = np.ascontiguousarray((-2.0 * A).T.astype(bf16))        # [D, N]
    btT = np.ascontiguousarray(Bp[:NPW].T.astype(bf16))          # [D, NPW]
    b2bf = b2p[:NPW].astype(bf16)
    bt_chunks = btT.reshape(KCH, 128, NPW)
    in_maps = []
    for c in range(NCORES):
        at_c = np.ascontiguousarray(
            atT[:, c * RPC:(c + 1) * RPC].reshape(KCH, 128, RPC)
        )
        a2_c = np.ascontiguousarray(
            a2[c * RPC:(c + 1) * RPC].reshape(MB, 128).T
        )
        in_maps.append({
            "at0": at_c,
            "bt0": bt_chunks,
            "b20": b2bf.reshape(1, NPW),
            "a20": a2_c,
        })

    trace = bool(int(os.environ.get("NN_COV_TRACE", "0")))
    LAST_RESULTS = run_bass_kernel_spmd(
        nc, in_maps, core_ids=list(range(NCORES)), trace=trace
    )
    results = LAST_RESULTS.results

    row_min = np.empty(N, dtype=np.float32)
    row_sum = np.empty(N, dtype=np.float32)
    for c in range(NCORES):
        row_min[c * RPC:(c + 1) * RPC] = results[c]["rmin0"].T.reshape(RPC)
        row_sum[c * RPC:(c + 1) * RPC] = results[c]["rsum0"].T.reshape(RPC)

    # Columns beyond the processed prefix have B == 0 exactly:
    # dist_ij = sqrt(max(a2_i, eps)). Add them in closed form.
    n_rest = N - NPW
    if n_rest > 0:
        row_sum = row_sum + np.float32(n_rest) * np.sqrt(
            np.maximum(a2, np.float32(EPS))
        ).astype(np.float32)

    norm = np.float32(row_sum.sum(dtype=np.float64) / (float(N) * float(N)))

    # Patch the diagonal with its exact value: d2_ii = (mq XOR ms) * s2_i.
    # (The device's diag entry carries matmul cancellation noise; the true
    # value is exact in closed form since A_i and B_i share seq_i.)
    d2_diag = np.where(mq ^ ms, s2, np.float32(0.0)).astype(np.float32)
    min_d2 = np.where(ms, np.minimum(row_min, d2_diag), row_min)
    if NV == 0:
        mn = np.full(N, np.inf, dtype=np.float32)
    else:
        mn = np.sqrt(np.maximum(min_d2, np.float32(EPS)))
    mn = np.minimum(mn, norm)
    simcov = (np.float32(1.0) - mn / norm).astype(np.float32)[:, None]
    out = simcov @ weight + bias[None, :]
    return out.astype(np.float32)
